# revision 8
# baseline (speedup 1.0000x reference)
"""Biased multi-head attention on 8 Trainium2 NeuronCores.

Sharding: batch x head-group. Core c handles batch b = c//4 and heads
4*(c%4) .. 4*(c%4)+3 (4 of 16 heads). Q/K/V projections are column-sharded
over the core's heads, scores/softmax/AV are fully local per head, and the
output projection is row-sharded (each core contributes a partial [D, L]
that the host sums per batch).

Key-side compaction: keys at padded positions get softmax weight ~0 in the
reference (score -1e4), so the host drops them up front; x / attn_bias are
compacted to the nk unpadded keys (padded up to Kp = ceil(nk/384)*384
slots). Slack slots are killed by the bias multiply (bias weight 0 there).

The kernel is paced by the ACT engine's exp stream (~1.15us per
(qt, kt, pair) step, 72 steps); everything else hides under it:
  - attn_bias ships fp16 exp(bias - ln4) in a DMA-swizzled layout
    [qt, kt, p, h, q] (2KB contiguous per partition per kt chunk) and
    loads on the gpsimd SWDGE queue so it never queues behind the x/W
    loads on the sync HWDGE queue.
  - x / W ship bf16 (same PE rate, half the DMA); out partials ship fp16.
  - scores are computed transposed, S_T[k, q] = kT.T @ qT per head
    (contraction 64); head pairs sit on partitions 0:64 / 64:128 so the
    two scores matmuls run concurrently on disjoint PE row groups.
  - ACT: ex1 = exp(S_T - ln8) (fp16); ex2 = ex1 * bias multiplies
    alternate between DVE and gpsimd (both SBUF-only).
  - AV: lhsT = [V | 1] fp16, rhs = ex2; accumulates O_T[c, q] and Z[q]
    (row 64) in one PSUM group per head pair.
  - softmax denominator: Z row 64 is broadcast to 64 partitions with a
    tiny ones-matmul (contraction 1 at partition 64) straight into a
    PSUM scores-pool tile; 1/Z on DVE from PSUM; O_T scaled into bf16
    otl quadrants (DVE writes with a partition offset for odd heads).
  - the out-projection for q-block qt runs interleaved inside qt+1's
    stream (two j-blocks per kt step at kt=5..8) in bf16, with the
    output bias applied as one broadcast DVE add per j-pair.
"""

import os

import numpy as np
import ml_dtypes

B, L, D, H = 2, 2048, 1024, 16
dh = D // H          # 64
NCORES = 8
HPC = 4              # heads per core
P = 128
LN8 = float(np.log(8.0))
LN4 = float(np.log(4.0))

_compiled = None     # (Kp, nc): compiled module and its key-slot count
LAST_RESULT = None   # BassKernelResults of the most recent run (for profiling)


def _build(Kp):
    from contextlib import ExitStack

    import concourse.bass as bass
    import concourse.tile as tile
    from concourse import bacc, mybir
    from concourse.bass import ts

    f32 = mybir.dt.float32
    bf16 = mybir.dt.bfloat16
    f16 = mybir.dt.float16
    Act = mybir.ActivationFunctionType
    KT = Kp // P          # 128-wide key chunks
    KT3 = Kp // 384       # 384-wide key tiles for the k/v projections

    nc = bacc.Bacc("TRN2", target_bir_lowering=False, debug=False,
                   num_devices=NCORES)

    xT_d = nc.dram_tensor("xT", [D, L], bf16, kind="ExternalInput").ap()
    xkT_d = nc.dram_tensor("xkT", [D, Kp], bf16, kind="ExternalInput").ap()
    wqkT_d = nc.dram_tensor("wqkT", [D, 512], bf16, kind="ExternalInput").ap()
    wvT_d = nc.dram_tensor("wvT", [D, 256], bf16, kind="ExternalInput").ap()
    bqk_d = nc.dram_tensor("bqk", [512], f32, kind="ExternalInput").ap()
    bias8_d = nc.dram_tensor("bias8", [4, KT, P, HPC, 512], f16,
                             kind="ExternalInput").ap()
    woutT_d = nc.dram_tensor("woutT", [256, D], bf16, kind="ExternalInput").ap()
    bout_d = nc.dram_tensor("bout4", [D], f32, kind="ExternalInput").ap()
    outT_d = nc.dram_tensor("outT", [D, L], f16, kind="ExternalOutput").ap()

    with tile.TileContext(nc) as tc, ExitStack() as ctx:
        consts = ctx.enter_context(tc.tile_pool(name="consts", bufs=1))
        xp = ctx.enter_context(tc.tile_pool(name="xp", bufs=2))
        biasp = ctx.enter_context(tc.tile_pool(name="biasp", bufs=2))
        expp = ctx.enter_context(tc.tile_pool(name="expp", bufs=3))
        avcp = ctx.enter_context(tc.tile_pool(name="avcp", bufs=1))
        zp = ctx.enter_context(tc.tile_pool(name="zp", bufs=2))
        otlp = ctx.enter_context(tc.tile_pool(name="otlp", bufs=2))
        outp = ctx.enter_context(tc.tile_pool(name="outp", bufs=2))
        psS = ctx.enter_context(tc.tile_pool(name="psS", bufs=2, space="PSUM"))
        psAV = ctx.enter_context(tc.tile_pool(name="psAV", bufs=2, space="PSUM"))

        # ---- prologue DMAs, first-needed first --------------------------
        xs0 = xp.tile([P, 8, 512], bf16, name="xs", tag="xs")
        nc.sync.dma_start(
            xs0, xT_d[:, 0:512].rearrange("(o p) l -> p o l", p=P))
        wqkT_sb = consts.tile([P, 8, 512], bf16, name="wqkT_sb", tag="wqkT_sb")
        for half in range(2):
            nc.sync.dma_start(
                wqkT_sb[:, 4 * half:4 * half + 4, :],
                wqkT_d[512 * half:512 * half + 512]
                .rearrange("(o p) m -> p o m", p=P))
        xks0 = xp.tile([P, 8, 384], bf16, name="xks", tag="xks")
        nc.sync.dma_start(
            xks0, xkT_d[:, 0:384].rearrange("(o p) l -> p o l", p=P))
        wvT_sb = consts.tile([P, 8, 256], bf16, name="wvT_sb", tag="wvT_sb")
        nc.sync.dma_start(wvT_sb, wvT_d.rearrange("(o p) m -> p o m", p=P))
        bqk_sb = consts.tile([P, 4], f32, name="bqk_sb", tag="bqk_sb")
        nc.sync.dma_start(bqk_sb, bqk_d.rearrange("(o p) -> p o", p=P))

        btws = {}

        def emit_bias_dma(qt):
            # SWDGE (gpsimd) queue: runs in parallel with the sync-queue
            # x/W loads instead of FIFO-ing behind them
            btw = biasp.tile([P, KT, HPC, 512], f16, name="btw", tag="btw")
            btws[qt] = btw
            for third in range(3):
                k0, k1 = third * (KT // 3), (third + 1) * (KT // 3)
                nc.gpsimd.dma_start(
                    btw[:, k0:k1, :, :],
                    bias8_d[qt, k0:k1].rearrange("kt p h q -> p kt h q"),
                )

        emit_bias_dma(0)

        woutT_sb = consts.tile([P, 2, D], bf16, name="woutT_sb", tag="woutT_sb")
        nc.sync.dma_start(woutT_sb, woutT_d.rearrange("(o p) m -> p o m", p=P))
        bout_sb = consts.tile([P, 8], f32, name="bout_sb", tag="bout_sb")
        nc.sync.dma_start(bout_sb, bout_d.rearrange("(o p) -> p o", p=P))

        qT_sb = consts.tile([P, 2, L], f16, name="qT_sb", tag="qT_sb")
        kT_sb = consts.tile([P, 2, Kp], f16, name="kT_sb", tag="kT_sb")
        V_sb = consts.tile([P, KT, HPC, 65], f16, name="V_sb", tag="V_sb")

        ones_c = consts.tile([P, 64], f32, name="ones_c", tag="ones_c")
        nc.vector.memset(ones_c, 1.0)
        mln8_c = consts.tile([P, 1], f32, name="mln8_c", tag="mln8_c")
        nc.vector.memset(mln8_c, -LN8)
        nc.vector.tensor_copy(
            V_sb[:, :, :, 64:65],
            ones_c[:, 0:1, None, None].to_broadcast((P, KT, HPC, 1)),
        )

        # ---- projections ------------------------------------------------
        def emit_qproj(lt):
            if lt == 0:
                xs = xs0
            else:
                xs = xp.tile([P, 8, 512], bf16, name="xs", tag="xs")
                nc.sync.dma_start(
                    xs, xT_d[:, ts(lt, 512)].rearrange("(o p) l -> p o l", p=P))
            ps = psS.tile([P, 2, 512], f32, name="ps_s", tag="ps_s")
            for rt in range(2):
                for dc in range(8):
                    nc.tensor.matmul(
                        ps[:, rt, :],
                        lhsT=wqkT_sb[:, dc, ts(rt, P)],
                        rhs=xs[:, dc, :],
                        start=(dc == 0), stop=(dc == 7),
                    )
            nc.vector.tensor_add(
                qT_sb[:, :, ts(lt, 512)], ps,
                bqk_sb[:, 0:2, None].to_broadcast((P, 2, 512)))

        def emit_kvproj(kt3):
            if kt3 == 0:
                xks = xks0
            else:
                xks = xp.tile([P, 8, 384], bf16, name="xks", tag="xks")
                nc.sync.dma_start(
                    xks,
                    xkT_d[:, ts(kt3, 384)].rearrange("(o p) l -> p o l", p=P))
            ps = psS.tile([P, 2, 512], f32, name="ps_s", tag="ps_s")
            for rt in range(2):
                for dc in range(8):
                    nc.tensor.matmul(
                        ps[:, rt, :384],
                        lhsT=wqkT_sb[:, dc, ts(2 + rt, P)],
                        rhs=xks[:, dc, :],
                        start=(dc == 0), stop=(dc == 7),
                    )
            nc.vector.tensor_add(
                kT_sb[:, :, ts(kt3, 384)], ps[:, :, :384],
                bqk_sb[:, 2:4, None].to_broadcast((P, 2, 384)))
            for l4 in range(3):
                ltv = kt3 * 3 + l4
                psv = psS.tile([P, 2, 512], f32, name="ps_s", tag="ps_s")[:, 0, :256]
                for dc in range(8):
                    nc.tensor.matmul(
                        psv,
                        lhsT=xks[:, dc, ts(l4, P)],
                        rhs=wvT_sb[:, dc, :],
                        start=(dc == 0), stop=(dc == 7),
                    )
                nc.vector.tensor_copy(
                    V_sb[:, ltv, :, 0:64],
                    psv.rearrange("p (h c) -> p h c", c=64),
                )

        # ---- attention stream -------------------------------------------
        avs = {}

        def emit_step(qt, kt, pair):
            swide = psS.tile([P, 2, 512], f32, name="swide", tag="ps_s")
            for hi in range(2):
                cs = slice(64 * hi, 64 * hi + 64)
                nc.tensor.matmul(
                    swide[:, hi, :],
                    lhsT=kT_sb[cs, pair, ts(kt, P)],
                    rhs=qT_sb[cs, pair, ts(qt, 512)],
                    start=True, stop=True,
                )
            ex1 = expp.tile([P, 2, 512], f16, name="ex1", tag="ex1")
            nc.scalar.activation(ex1, swide, Act.Exp, bias=mln8_c[:, 0:1],
                                 scale=1.0)
            ex2 = expp.tile([P, 2, 512], f16, name="ex2", tag="ex2")
            eng = nc.vector if pair == 0 else nc.gpsimd
            eng.tensor_mul(
                ex2, ex1, btws[qt][:, kt, 2 * pair:2 * pair + 2, :])
            for hi in range(2):
                h = 2 * pair + hi
                nc.tensor.matmul(
                    avs[qt][pair][:, hi, :],
                    lhsT=V_sb[:, kt, h, :],
                    rhs=ex2[:, hi, :],
                    start=(kt == 0), stop=(kt == KT - 1),
                )

        avcs = {}

        def emit_evac(qt):
            avc = avcp.tile([65, HPC, 512], f32, name="avc", tag="avc")
            avcs[qt] = avc
            nc.vector.tensor_copy(avc[:, 0:2, :], avs[qt][0])
            nc.vector.tensor_copy(avc[:, 2:4, :], avs[qt][1])

        def emit_norm(qt):
            # broadcast Z (avc row 64) to 64 partitions with a
            # contraction-1 ones-matmul, reciprocal from PSUM on DVE,
            # then scale O_T into bf16 otl quadrants
            avc = avcs[qt]
            otl = otlp.tile([P, 2, 512], bf16, name="otl", tag="otl")
            for pair in range(2):
                zbc = psS.tile([P, 2, 512], f32, name="zbc", tag="ps_s")
                for hi in range(2):
                    nc.tensor.matmul(
                        zbc[0:64, hi, :],
                        lhsT=ones_c[64:65, :],
                        rhs=avc[64:65, 2 * pair + hi, :],
                        start=True, stop=True,
                    )
                zrt = zp.tile([64, 2, 512], f32, name="zrt", tag="zrt")
                nc.vector.reciprocal_approx_fast(zrt, zbc[0:64, :, :])
                for hi in range(2):
                    h = 2 * pair + hi
                    nc.vector.tensor_mul(
                        otl[64 * hi:64 * hi + 64, pair, :],
                        avc[0:64, h, :], zrt[:, hi, :])
            return otl

        otls = {}

        def emit_proj_pair(qt, jp):
            # out-projection for j-blocks 2*jp, 2*jp+1 of q-block qt
            otl = otls[qt]
            ps = psS.tile([P, 2, 512], f32, name="ps_s", tag="ps_s")
            for j2 in range(2):
                jt = 2 * jp + j2
                for cc in range(2):
                    nc.tensor.matmul(
                        ps[:, j2, :],
                        lhsT=woutT_sb[:, cc, ts(jt, P)],
                        rhs=otl[:, cc, :],
                        start=(cc == 0), stop=(cc == 1),
                    )
            osb = osbs[qt]
            nc.vector.tensor_add(
                osb[:, 2 * jp:2 * jp + 2, :], ps,
                bout_sb[:, 2 * jp:2 * jp + 2, None].to_broadcast((P, 2, 512)))
            if jp == 3:
                nc.sync.dma_start(
                    outT_d[:, ts(qt, 512)].rearrange("(o p) l -> p o l", p=P),
                    osb)

        osbs = {}

        # ---- schedule ---------------------------------------------------
        avs[0] = [psAV.tile([65, 2, 512], f32, name=f"av0{pp}", tag="av")
                  for pp in range(2)]
        emit_qproj(0)
        emit_kvproj(0)
        for kt3 in range(KT3):
            if kt3 + 1 < KT3:
                emit_kvproj(kt3 + 1)
            if kt3 + 1 < 4:
                emit_qproj(kt3 + 1)
            for kt in range(3 * kt3, min(3 * kt3 + 3, KT)):
                for pair in range(2):
                    emit_step(0, kt, pair)
        for lt in range(KT3 + 1, 4):
            emit_qproj(lt)
        emit_evac(0)
        for qt in range(1, 4):
            emit_bias_dma(qt)
            avs[qt] = [psAV.tile([65, 2, 512], f32, name=f"av{qt}{pp}", tag="av")
                       for pp in range(2)]
            osbs[qt - 1] = outp.tile([P, 8, 512], f16, name="osb", tag="osb")
            for kt in range(KT):
                for pair in range(2):
                    emit_step(qt, kt, pair)
                if kt == 0:
                    otls[qt - 1] = emit_norm(qt - 1)
                if 5 <= kt <= 8:
                    emit_proj_pair(qt - 1, kt - 5)
            emit_evac(qt)
        osbs[3] = outp.tile([P, 8, 512], f16, name="osb", tag="osb")
        otls[3] = emit_norm(3)
        for jp in range(4):
            emit_proj_pair(3, jp)

    nc.compile()
    return nc


def _prep_core_inputs(c, Kp, x, key_padding_mask, attn_bias, W_in, b_in,
                      W_out, b_out):
    b, hg = c // HPC, c % HPC
    hs = slice(256 * hg, 256 * hg + 256)
    f32 = np.float32
    bf16 = ml_dtypes.bfloat16
    KT = Kp // P
    idx = np.where(~key_padding_mask[b])[0]
    nk = len(idx)
    wq, wk, wv = W_in[0:D][hs], W_in[D:2 * D][hs], W_in[2 * D:3 * D][hs]

    xk = np.zeros((Kp, D), dtype=f32)
    xk[:nk] = x[b][idx]

    # bias weights: exp(bias - ln4) fp16, zero at slack slots so the
    # multiply masks them; swizzled [qt, kt, p, h, q] so each partition's
    # DMA read is contiguous per kt chunk.
    full = np.zeros((HPC, Kp, L), dtype=f32)
    full[:, :nk, :] = np.exp(
        attn_bias[b, HPC * hg:HPC * hg + HPC][:, :, idx]
        .transpose(0, 2, 1) - LN4)
    bias8 = np.ascontiguousarray(
        full.reshape(HPC, KT, P, 4, 512).transpose(3, 1, 2, 0, 4)
    ).astype(np.float16)

    return {
        "xT": np.ascontiguousarray(x[b].T).astype(bf16),
        "xkT": np.ascontiguousarray(xk.T).astype(bf16),
        "wqkT": np.ascontiguousarray(
            np.concatenate([wq / 8.0, wk], 0).T).astype(bf16),
        "wvT": np.ascontiguousarray(wv.T).astype(bf16),
        "bqk": np.concatenate([b_in[0:D][hs] / 8.0, b_in[D:2 * D][hs]]).astype(f32),
        "bias8": bias8,
        "woutT": np.ascontiguousarray(W_out[:, hs].T).astype(bf16),
        "bout4": (b_out / float(HPC) + W_out[:, hs] @ b_in[2 * D:3 * D][hs]).astype(f32),
    }


def kernel(x, key_padding_mask, attn_bias, W_in, b_in, W_out, b_out):
    global _compiled, LAST_RESULT
    from concourse.bass_utils import run_bass_kernel_spmd

    nk_max = int((~key_padding_mask).sum(axis=1).max())
    Kp = max(384, -(-nk_max // 384) * 384)

    if _compiled is None or _compiled[0] != Kp:
        _compiled = (Kp, _build(Kp))

    in_maps = [
        _prep_core_inputs(c, Kp, x, key_padding_mask, attn_bias, W_in, b_in,
                          W_out, b_out)
        for c in range(NCORES)
    ]
    res = run_bass_kernel_spmd(
        _compiled[1], in_maps, core_ids=list(range(NCORES)),
        trace_cores=(list(range(NCORES))
                     if os.environ.get("BASS_TRACE") == "1" else None),
    )
    LAST_RESULT = res

    out = np.empty((B, L, D), dtype=np.float32)
    for b in range(B):
        acc = res.results[b * HPC]["outT"].astype(np.float64)
        for g in range(1, HPC):
            acc = acc + res.results[b * HPC + g]["outT"]
        out[b] = acc.T.astype(np.float32)
    return out


# revision 9
# speedup vs baseline: 1.1087x; 1.1087x over previous
"""Biased multi-head attention on 8 Trainium2 NeuronCores.

Sharding: batch x head-group. Core c handles batch b = c//4 and heads
4*(c%4) .. 4*(c%4)+3 (4 of 16 heads). Q/K/V projections are column-sharded
over the core's heads, scores/softmax/AV are fully local per head, and the
output projection is row-sharded (each core contributes a partial [D, L]
that the host sums per batch).

Key-side compaction: keys at padded positions get softmax weight ~0 in the
reference (score -1e4), so the host drops them up front; x / attn_bias are
compacted to the nk unpadded keys (padded up to Kp = ceil(nk/384)*384
slots). Slack slots are killed by the bias multiply (bias weight 0 there).

The kernel is paced by the ACT engine's exp stream (~1.15us per
(qt, kt, pair) step, 72 steps); everything else hides under it:
  - attn_bias ships fp16 exp(bias - ln4) in a DMA-swizzled layout
    [qt, kt, p, h, q] (2KB contiguous per partition per kt chunk) and
    loads on the gpsimd SWDGE queue so it never queues behind the x/W
    loads on the sync HWDGE queue.
  - x / W ship bf16 (same PE rate, half the DMA); out partials ship fp16.
  - scores are computed transposed, S_T[k, q] = kT.T @ qT per head
    (contraction 64); head pairs sit on partitions 0:64 / 64:128 so the
    two scores matmuls run concurrently on disjoint PE row groups.
  - ACT: ex1 = exp(S_T - ln8) (fp16); ex2 = ex1 * bias multiplies
    alternate between DVE and gpsimd (both SBUF-only).
  - AV: lhsT = [V | 1] fp16, rhs = ex2; accumulates O_T[c, q] and Z[q]
    (row 64) in one PSUM group per head pair.
  - softmax denominator: Z row 64 is broadcast to 64 partitions with a
    tiny ones-matmul (contraction 1 at partition 64) straight into a
    PSUM scores-pool tile; 1/Z on DVE from PSUM; O_T scaled into bf16
    otl quadrants (DVE writes with a partition offset for odd heads).
  - the out-projection for q-block qt runs interleaved inside qt+1's
    stream (two j-blocks per kt step at kt=5..8) in bf16, with the
    output bias applied as one broadcast DVE add per j-pair.
"""

import os

import numpy as np
import ml_dtypes

B, L, D, H = 2, 2048, 1024, 16
dh = D // H          # 64
NCORES = 8
HPC = 4              # heads per core
P = 128
LN8 = float(np.log(8.0))
LN4 = float(np.log(4.0))

_compiled = None     # (Kp, nc): compiled module and its key-slot count
LAST_RESULT = None   # BassKernelResults of the most recent run (for profiling)


def _build(Kp):
    from contextlib import ExitStack

    import concourse.bass as bass
    import concourse.tile as tile
    from concourse import bacc, mybir
    from concourse.bass import ts

    f32 = mybir.dt.float32
    bf16 = mybir.dt.bfloat16
    f16 = mybir.dt.float16
    Act = mybir.ActivationFunctionType
    KT = Kp // P          # 128-wide key chunks
    KT3 = Kp // 384       # 384-wide key tiles for the k/v projections

    nc = bacc.Bacc("TRN2", target_bir_lowering=False, debug=False,
                   num_devices=NCORES)

    xT_d = nc.dram_tensor("xT", [D, L], bf16, kind="ExternalInput").ap()
    xkT_d = nc.dram_tensor("xkT", [D, Kp], bf16, kind="ExternalInput").ap()
    wqkT_d = nc.dram_tensor("wqkT", [D, 512], bf16, kind="ExternalInput").ap()
    wvT_d = nc.dram_tensor("wvT", [D, 256], bf16, kind="ExternalInput").ap()
    bqk_d = nc.dram_tensor("bqk", [512], f32, kind="ExternalInput").ap()
    bias8_d = nc.dram_tensor("bias8", [4, KT, P, HPC, 512], f16,
                             kind="ExternalInput").ap()
    woutT_d = nc.dram_tensor("woutT", [256, D], bf16, kind="ExternalInput").ap()
    bout_d = nc.dram_tensor("bout4", [D], f32, kind="ExternalInput").ap()
    outT_d = nc.dram_tensor("outT", [D, L], f16, kind="ExternalOutput").ap()

    with tile.TileContext(nc) as tc, ExitStack() as ctx:
        consts = ctx.enter_context(tc.tile_pool(name="consts", bufs=1))
        xp = ctx.enter_context(tc.tile_pool(name="xp", bufs=2))
        biasp = ctx.enter_context(tc.tile_pool(name="biasp", bufs=2))
        expp = ctx.enter_context(tc.tile_pool(name="expp", bufs=3))
        zp = ctx.enter_context(tc.tile_pool(name="zp", bufs=2))
        otlp = ctx.enter_context(tc.tile_pool(name="otlp", bufs=2))
        outp = ctx.enter_context(tc.tile_pool(name="outp", bufs=2))
        psS = ctx.enter_context(tc.tile_pool(name="psS", bufs=2, space="PSUM"))
        psAV = ctx.enter_context(tc.tile_pool(name="psAV", bufs=2, space="PSUM"))

        # ---- prologue DMAs, first-needed first --------------------------
        xs0 = xp.tile([P, 8, 512], bf16, name="xs", tag="xs")
        nc.sync.dma_start(
            xs0, xT_d[:, 0:512].rearrange("(o p) l -> p o l", p=P))
        wqkT_sb = consts.tile([P, 8, 512], bf16, name="wqkT_sb", tag="wqkT_sb")
        for half in range(2):
            nc.sync.dma_start(
                wqkT_sb[:, 4 * half:4 * half + 4, :],
                wqkT_d[512 * half:512 * half + 512]
                .rearrange("(o p) m -> p o m", p=P))
        xks0 = xp.tile([P, 8, 384], bf16, name="xks", tag="xks")
        nc.sync.dma_start(
            xks0, xkT_d[:, 0:384].rearrange("(o p) l -> p o l", p=P))
        wvT_sb = consts.tile([P, 8, 256], bf16, name="wvT_sb", tag="wvT_sb")
        nc.sync.dma_start(wvT_sb, wvT_d.rearrange("(o p) m -> p o m", p=P))
        bqk_sb = consts.tile([P, 4], f32, name="bqk_sb", tag="bqk_sb")
        nc.sync.dma_start(bqk_sb, bqk_d.rearrange("(o p) -> p o", p=P))

        btws = {}

        def emit_bias_dma(qt):
            # ACT's HWDGE ring (qActDynamicHW): parallel FIFO to the
            # sync ring so the bias never queues behind the x/W loads
            btw = biasp.tile([P, KT, HPC, 512], f16, name="btw", tag="btw")
            btws[qt] = btw
            for third in range(3):
                k0, k1 = third * (KT // 3), (third + 1) * (KT // 3)
                nc.scalar.dma_start(
                    btw[:, k0:k1, :, :],
                    bias8_d[qt, k0:k1].rearrange("kt p h q -> p kt h q"),
                )

        emit_bias_dma(0)

        woutT_sb = consts.tile([P, 2, D], bf16, name="woutT_sb", tag="woutT_sb")
        nc.sync.dma_start(woutT_sb, woutT_d.rearrange("(o p) m -> p o m", p=P))
        bout_sb = consts.tile([P, 8], f32, name="bout_sb", tag="bout_sb")
        nc.sync.dma_start(bout_sb, bout_d.rearrange("(o p) -> p o", p=P))

        qT_sb = consts.tile([P, 2, L], f16, name="qT_sb", tag="qT_sb")
        kT_sb = consts.tile([P, 2, Kp], f16, name="kT_sb", tag="kT_sb")
        V_sb = consts.tile([P, KT, HPC, 65], f16, name="V_sb", tag="V_sb")

        ones_c = consts.tile([P, 64], f32, name="ones_c", tag="ones_c")
        nc.vector.memset(ones_c, 1.0)
        mln8_c = consts.tile([P, 1], f32, name="mln8_c", tag="mln8_c")
        nc.vector.memset(mln8_c, -LN8)
        nc.vector.tensor_copy(
            V_sb[:, :, :, 64:65],
            ones_c[:, 0:1, None, None].to_broadcast((P, KT, HPC, 1)),
        )

        # ---- projections ------------------------------------------------
        def emit_qproj(lt):
            if lt == 0:
                xs = xs0
            else:
                xs = xp.tile([P, 8, 512], bf16, name="xs", tag="xs")
                nc.sync.dma_start(
                    xs, xT_d[:, ts(lt, 512)].rearrange("(o p) l -> p o l", p=P))
            ps = psS.tile([P, 2, 512], f32, name="ps_s", tag="ps_s")
            for rt in range(2):
                for dc in range(8):
                    nc.tensor.matmul(
                        ps[:, rt, :],
                        lhsT=wqkT_sb[:, dc, ts(rt, P)],
                        rhs=xs[:, dc, :],
                        start=(dc == 0), stop=(dc == 7),
                    )
            for rt in range(2):
                nc.scalar.activation(
                    qT_sb[:, rt, ts(lt, 512)], ps[:, rt, :], Act.Identity,
                    bias=bqk_sb[:, rt:rt + 1], scale=1.0)

        def emit_kvproj(kt3):
            if kt3 == 0:
                xks = xks0
            else:
                xks = xp.tile([P, 8, 384], bf16, name="xks", tag="xks")
                nc.sync.dma_start(
                    xks,
                    xkT_d[:, ts(kt3, 384)].rearrange("(o p) l -> p o l", p=P))
            ps = psS.tile([P, 2, 512], f32, name="ps_s", tag="ps_s")
            for rt in range(2):
                for dc in range(8):
                    nc.tensor.matmul(
                        ps[:, rt, :384],
                        lhsT=wqkT_sb[:, dc, ts(2 + rt, P)],
                        rhs=xks[:, dc, :],
                        start=(dc == 0), stop=(dc == 7),
                    )
            for rt in range(2):
                nc.scalar.activation(
                    kT_sb[:, rt, ts(kt3, 384)], ps[:, rt, :384], Act.Identity,
                    bias=bqk_sb[:, 2 + rt:3 + rt], scale=1.0)
            for l4 in range(3):
                ltv = kt3 * 3 + l4
                psv = psS.tile([P, 2, 512], f32, name="ps_s", tag="ps_s")[:, 0, :256]
                for dc in range(8):
                    nc.tensor.matmul(
                        psv,
                        lhsT=xks[:, dc, ts(l4, P)],
                        rhs=wvT_sb[:, dc, :],
                        start=(dc == 0), stop=(dc == 7),
                    )
                nc.vector.tensor_copy(
                    V_sb[:, ltv, :, 0:64],
                    psv.rearrange("p (h c) -> p h c", c=64),
                )

        # ---- attention stream -------------------------------------------
        avs = {}

        def emit_scores(qt, kt, pair):
            swide = psS.tile([P, 2, 512], f32, name="swide", tag="ps_s")
            for hi in range(2):
                cs = slice(64 * hi, 64 * hi + 64)
                nc.tensor.matmul(
                    swide[:, hi, :],
                    lhsT=kT_sb[cs, pair, ts(kt, P)],
                    rhs=qT_sb[cs, pair, ts(qt, 512)],
                    start=True, stop=True,
                )
            ex1 = expp.tile([P, 2, 512], f16, name="ex1", tag="ex1")
            nc.scalar.activation(ex1, swide, Act.Exp, bias=mln8_c[:, 0:1],
                                 scale=1.0)
            ex2 = expp.tile([P, 2, 512], f16, name="ex2", tag="ex2")
            nc.vector.tensor_mul(
                ex2, ex1, btws[qt][:, kt, 2 * pair:2 * pair + 2, :])
            return ex2

        def emit_av(qt, kt, pair, ex2):
            for hi in range(2):
                h = 2 * pair + hi
                nc.tensor.matmul(
                    avs[qt][pair][:, hi, :],
                    lhsT=V_sb[:, kt, h, :],
                    rhs=ex2[:, hi, :],
                    start=(kt == 0), stop=(kt == KT - 1),
                )

        def emit_step(qt, kt, pair):
            emit_av(qt, kt, pair, emit_scores(qt, kt, pair))

        zrows = {}

        def emit_zrow(qt):
            # Z rows (PSUM row 64 of each AV pair) -> one tiny SBUF tile
            zrow = zp.tile([1, HPC, 512], f32, name="zrow", tag="zrow")
            zrows[qt] = zrow
            nc.vector.tensor_copy(zrow[0:1, 0:2, :], avs[qt][0][64:65, :, :])
            nc.vector.tensor_copy(zrow[0:1, 2:4, :], avs[qt][1][64:65, :, :])

        def emit_norm(qt):
            # broadcast raw Z to 64 partitions with a contraction-1
            # ones-matmul; the reciprocal doubles as the PSUM evacuation;
            # otl muls read the AV PSUM directly (one-PSUM-operand rule)
            zrow = zrows[qt]
            otl = otlp.tile([P, 2, 512], bf16, name="otl", tag="otl")
            for pair in range(2):
                zbc = psS.tile([P, 2, 512], f32, name="zbc", tag="ps_s")
                for hi in range(2):
                    nc.tensor.matmul(
                        zbc[0:64, hi, :],
                        lhsT=ones_c[0:1, :],
                        rhs=zrow[0:1, 2 * pair + hi, :],
                        start=True, stop=True,
                    )
                zrt = zp.tile([64, 2, 512], f32, name="zrt", tag="zrt")
                nc.vector.reciprocal_approx_fast(zrt, zbc[0:64, :, :])
                for hi in range(2):
                    nc.vector.tensor_mul(
                        otl[64 * hi:64 * hi + 64, pair, :],
                        avs[qt][pair][0:64, hi, :], zrt[:, hi, :])
            return otl

        otls = {}

        def emit_proj_pair(qt, jp):
            # out-projection for j-blocks 2*jp, 2*jp+1 of q-block qt
            otl = otls[qt]
            ps = psS.tile([P, 2, 512], f32, name="ps_s", tag="ps_s")
            for j2 in range(2):
                jt = 2 * jp + j2
                for cc in range(2):
                    nc.tensor.matmul(
                        ps[:, j2, :],
                        lhsT=woutT_sb[:, cc, ts(jt, P)],
                        rhs=otl[:, cc, :],
                        start=(cc == 0), stop=(cc == 1),
                    )
            osb = osbs[qt]
            nc.vector.tensor_add(
                osb[:, 2 * jp:2 * jp + 2, :], ps,
                bout_sb[:, 2 * jp:2 * jp + 2, None].to_broadcast((P, 2, 512)))
            if jp == 3:
                nc.sync.dma_start(
                    outT_d[:, ts(qt, 512)].rearrange("(o p) l -> p o l", p=P),
                    osb)

        osbs = {}

        # ---- schedule ---------------------------------------------------
        avs[0] = [psAV.tile([65, 2, 512], f32, name=f"av0{pp}", tag="av")
                  for pp in range(2)]
        emit_qproj(0)
        emit_kvproj(0)
        for kt3 in range(KT3):
            if kt3 + 1 < KT3:
                emit_kvproj(kt3 + 1)
            if kt3 + 1 < 4:
                emit_qproj(kt3 + 1)
            for kt in range(3 * kt3, min(3 * kt3 + 3, KT)):
                for pair in range(2):
                    emit_step(0, kt, pair)
        for lt in range(KT3 + 1, 4):
            emit_qproj(lt)
        for qt in range(1, 4):
            emit_bias_dma(qt)
            emit_zrow(qt - 1)
            avs[qt] = [psAV.tile([65, 2, 512], f32, name=f"av{qt}{pp}", tag="av")
                       for pp in range(2)]
            osbs[qt - 1] = outp.tile([P, 8, 512], f16, name="osb", tag="osb")
            ex0 = [emit_scores(qt, 0, pair) for pair in range(2)]
            otls[qt - 1] = emit_norm(qt - 1)
            ex1_ = [emit_scores(qt, 1, pair) for pair in range(2)]
            for pair in range(2):
                emit_av(qt, 0, pair, ex0[pair])
            for pair in range(2):
                emit_av(qt, 1, pair, ex1_[pair])
            for kt in range(2, KT):
                for pair in range(2):
                    emit_step(qt, kt, pair)
                if 5 <= kt <= 8:
                    emit_proj_pair(qt - 1, kt - 5)
        osbs[3] = outp.tile([P, 8, 512], f16, name="osb", tag="osb")
        emit_zrow(3)
        otls[3] = emit_norm(3)
        for jp in range(4):
            emit_proj_pair(3, jp)

    nc.compile()
    return nc


def _prep_core_inputs(c, Kp, x, key_padding_mask, attn_bias, W_in, b_in,
                      W_out, b_out):
    b, hg = c // HPC, c % HPC
    hs = slice(256 * hg, 256 * hg + 256)
    f32 = np.float32
    bf16 = ml_dtypes.bfloat16
    KT = Kp // P
    idx = np.where(~key_padding_mask[b])[0]
    nk = len(idx)
    wq, wk, wv = W_in[0:D][hs], W_in[D:2 * D][hs], W_in[2 * D:3 * D][hs]

    xk = np.zeros((Kp, D), dtype=f32)
    xk[:nk] = x[b][idx]

    # bias weights: exp(bias - ln4) fp16, zero at slack slots so the
    # multiply masks them; swizzled [qt, kt, p, h, q] so each partition's
    # DMA read is contiguous per kt chunk.
    full = np.zeros((HPC, Kp, L), dtype=f32)
    full[:, :nk, :] = np.exp(
        attn_bias[b, HPC * hg:HPC * hg + HPC][:, :, idx]
        .transpose(0, 2, 1) - LN4)
    bias8 = np.ascontiguousarray(
        full.reshape(HPC, KT, P, 4, 512).transpose(3, 1, 2, 0, 4)
    ).astype(np.float16)

    return {
        "xT": np.ascontiguousarray(x[b].T).astype(bf16),
        "xkT": np.ascontiguousarray(xk.T).astype(bf16),
        "wqkT": np.ascontiguousarray(
            np.concatenate([wq / 8.0, wk], 0).T).astype(bf16),
        "wvT": np.ascontiguousarray(wv.T).astype(bf16),
        "bqk": np.concatenate([b_in[0:D][hs] / 8.0, b_in[D:2 * D][hs]]).astype(f32),
        "bias8": bias8,
        "woutT": np.ascontiguousarray(W_out[:, hs].T).astype(bf16),
        "bout4": (b_out / float(HPC) + W_out[:, hs] @ b_in[2 * D:3 * D][hs]).astype(f32),
    }


def kernel(x, key_padding_mask, attn_bias, W_in, b_in, W_out, b_out):
    global _compiled, LAST_RESULT
    from concourse.bass_utils import run_bass_kernel_spmd

    nk_max = int((~key_padding_mask).sum(axis=1).max())
    Kp = max(384, -(-nk_max // 384) * 384)

    if _compiled is None or _compiled[0] != Kp:
        _compiled = (Kp, _build(Kp))

    in_maps = [
        _prep_core_inputs(c, Kp, x, key_padding_mask, attn_bias, W_in, b_in,
                          W_out, b_out)
        for c in range(NCORES)
    ]
    res = run_bass_kernel_spmd(
        _compiled[1], in_maps, core_ids=list(range(NCORES)),
        trace_cores=(list(range(NCORES))
                     if os.environ.get("BASS_TRACE") == "1" else None),
    )
    LAST_RESULT = res

    out = np.empty((B, L, D), dtype=np.float32)
    for b in range(B):
        acc = res.results[b * HPC]["outT"].astype(np.float64)
        for g in range(1, HPC):
            acc = acc + res.results[b * HPC + g]["outT"]
        out[b] = acc.T.astype(np.float32)
    return out


# revision 11
# speedup vs baseline: 1.1751x; 1.0598x over previous
"""Biased multi-head attention on 8 Trainium2 NeuronCores.

Sharding: batch x head-group. Core c handles batch b = c//4 and heads
4*(c%4) .. 4*(c%4)+3 (4 of 16 heads). Q/K/V projections are column-sharded
over the core's heads, scores/softmax/AV are fully local per head, and the
output projection is row-sharded (each core contributes a partial [D, L]
that the host sums per batch).

Key-side compaction: keys at padded positions get softmax weight ~0 in the
reference (score -1e4), so the host drops them up front; x / attn_bias are
compacted to the nk unpadded keys (padded up to Kp = ceil(nk/384)*384
slots). Slack slots are killed by the bias multiply (bias weight 0 there).

The kernel is paced by the ACT engine's exp stream (~1.15us per
(qt, kt, pair) step, 72 steps); everything else hides under it:
  - attn_bias ships fp16 exp(bias - ln4) in a DMA-swizzled layout
    [qt, kt, p, h, q] (2KB contiguous per partition per kt chunk) and
    loads on the gpsimd SWDGE queue so it never queues behind the x/W
    loads on the sync HWDGE queue.
  - x / W ship bf16 (same PE rate, half the DMA); out partials ship fp16.
  - scores are computed transposed, S_T[k, q] = kT.T @ qT per head
    (contraction 64); head pairs sit on partitions 0:64 / 64:128 so the
    two scores matmuls run concurrently on disjoint PE row groups.
  - ACT: ex1 = exp(S_T - ln8) (fp16); ex2 = ex1 * bias multiplies
    alternate between DVE and gpsimd (both SBUF-only).
  - AV: lhsT = [V | 1] fp16, rhs = ex2; accumulates O_T[c, q] and Z[q]
    (row 64) in one PSUM group per head pair.
  - softmax denominator: Z row 64 is broadcast to 64 partitions with a
    tiny ones-matmul (contraction 1 at partition 64) straight into a
    PSUM scores-pool tile; 1/Z on DVE from PSUM; O_T scaled into bf16
    otl quadrants (DVE writes with a partition offset for odd heads).
  - the out-projection for q-block qt runs interleaved inside qt+1's
    stream (two j-blocks per kt step at kt=5..8) in bf16, with the
    output bias applied as one broadcast DVE add per j-pair.
"""

import os

import numpy as np
import ml_dtypes

B, L, D, H = 2, 2048, 1024, 16
dh = D // H          # 64
NCORES = 8
HPC = 4              # heads per core
P = 128
LN8 = float(np.log(8.0))
LN4 = float(np.log(4.0))

_compiled = None     # (Kp, nc): compiled module and its key-slot count
LAST_RESULT = None   # BassKernelResults of the most recent run (for profiling)


def _build(Kp):
    from contextlib import ExitStack

    import concourse.bass as bass
    import concourse.tile as tile
    from concourse import bacc, mybir
    from concourse.bass import ts

    f32 = mybir.dt.float32
    bf16 = mybir.dt.bfloat16
    f16 = mybir.dt.float16
    Act = mybir.ActivationFunctionType
    KT = Kp // P          # 128-wide key chunks
    KT3 = Kp // 384       # 384-wide key tiles for the k/v projections

    nc = bacc.Bacc("TRN2", target_bir_lowering=False, debug=False,
                   num_devices=NCORES)

    xT_d = nc.dram_tensor("xT", [D, L], bf16, kind="ExternalInput").ap()
    xkT_d = nc.dram_tensor("xkT", [D, Kp], bf16, kind="ExternalInput").ap()
    wqkT_d = nc.dram_tensor("wqkT", [D, 512], bf16, kind="ExternalInput").ap()
    wvT_d = nc.dram_tensor("wvT", [D, 256], bf16, kind="ExternalInput").ap()
    bqk_d = nc.dram_tensor("bqk", [512], f32, kind="ExternalInput").ap()
    bias8_d = nc.dram_tensor("bias8", [4, KT, P, HPC, 512], f16,
                             kind="ExternalInput").ap()
    woutT_d = nc.dram_tensor("woutT", [256, D], bf16, kind="ExternalInput").ap()
    bout_d = nc.dram_tensor("bout4", [D], f32, kind="ExternalInput").ap()
    outT_d = nc.dram_tensor("outT", [D, L], f16, kind="ExternalOutput").ap()

    with tile.TileContext(nc) as tc, ExitStack() as ctx:
        consts = ctx.enter_context(tc.tile_pool(name="consts", bufs=1))
        xp = ctx.enter_context(tc.tile_pool(name="xp", bufs=2))
        biasp = ctx.enter_context(tc.tile_pool(name="biasp", bufs=2))
        expp = ctx.enter_context(tc.tile_pool(name="expp", bufs=4))
        zp = ctx.enter_context(tc.tile_pool(name="zp", bufs=2))
        otlp = ctx.enter_context(tc.tile_pool(name="otlp", bufs=2))
        outp = ctx.enter_context(tc.tile_pool(name="outp", bufs=2))
        psS = ctx.enter_context(tc.tile_pool(name="psS", bufs=2, space="PSUM"))
        psAV = ctx.enter_context(tc.tile_pool(name="psAV", bufs=2, space="PSUM"))

        # ---- prologue DMAs, first-needed first --------------------------
        xs0 = xp.tile([P, 8, 512], bf16, name="xs", tag="xs")
        nc.sync.dma_start(
            xs0, xT_d[:, 0:512].rearrange("(o p) l -> p o l", p=P))
        wqkT_sb = consts.tile([P, 8, 512], bf16, name="wqkT_sb", tag="wqkT_sb")
        for half in range(2):
            nc.sync.dma_start(
                wqkT_sb[:, 4 * half:4 * half + 4, :],
                wqkT_d[512 * half:512 * half + 512]
                .rearrange("(o p) m -> p o m", p=P))
        xks0 = xp.tile([P, 8, 384], bf16, name="xks", tag="xks")
        nc.sync.dma_start(
            xks0, xkT_d[:, 0:384].rearrange("(o p) l -> p o l", p=P))
        wvT_sb = consts.tile([P, 8, 256], bf16, name="wvT_sb", tag="wvT_sb")
        nc.sync.dma_start(wvT_sb, wvT_d.rearrange("(o p) m -> p o m", p=P))
        bqk_sb = consts.tile([P, 4], f32, name="bqk_sb", tag="bqk_sb")
        nc.sync.dma_start(bqk_sb, bqk_d.rearrange("(o p) -> p o", p=P))

        btws = {}

        def emit_bias_dma(qt):
            # ACT's HWDGE ring (qActDynamicHW): parallel FIFO to the
            # sync ring so the bias never queues behind the x/W loads.
            # qt=0 loads kt-at-a-time so the stream can start on kt=0.
            btw = biasp.tile([P, KT, HPC, 512], f16, name="btw", tag="btw")
            btws[qt] = btw
            bounds = ([0, 1, 2, 3, 5, 7, KT] if qt == 0
                      else [0, 2, 4, 6, 8, KT])
            for k0, k1 in zip(bounds, bounds[1:]):
                nc.scalar.dma_start(
                    btw[:, k0:k1, :, :],
                    bias8_d[qt, k0:k1].rearrange("kt p h q -> p kt h q"),
                )

        emit_bias_dma(0)

        qT_sb = consts.tile([P, 2, L], f16, name="qT_sb", tag="qT_sb")
        kT_sb = consts.tile([P, 2, Kp], f16, name="kT_sb", tag="kT_sb")
        V_sb = consts.tile([P, KT, HPC, 65], f16, name="V_sb", tag="V_sb")

        ones_c = consts.tile([P, 64], f16, name="ones_c", tag="ones_c")
        nc.vector.memset(ones_c, 1.0)
        mln8_c = consts.tile([P, 1], f32, name="mln8_c", tag="mln8_c")
        nc.vector.memset(mln8_c, -LN8)
        nc.vector.tensor_copy(
            V_sb[:, :, :, 64:65],
            ones_c[:, 0:1, None, None].to_broadcast((P, KT, HPC, 1)),
        )

        # ---- projections ------------------------------------------------
        def emit_qproj(lt):
            if lt == 0:
                xs = xs0
            else:
                xs = xp.tile([P, 8, 512], bf16, name="xs", tag="xs")
                nc.sync.dma_start(
                    xs, xT_d[:, ts(lt, 512)].rearrange("(o p) l -> p o l", p=P))
            ps = psS.tile([P, 2, 512], f32, name="ps_s", tag="ps_s")
            for rt in range(2):
                for dc in range(8):
                    nc.tensor.matmul(
                        ps[:, rt, :],
                        lhsT=wqkT_sb[:, dc, ts(rt, P)],
                        rhs=xs[:, dc, :],
                        start=(dc == 0), stop=(dc == 7),
                    )
            for rt in range(2):
                nc.scalar.activation(
                    qT_sb[:, rt, ts(lt, 512)], ps[:, rt, :], Act.Identity,
                    bias=bqk_sb[:, rt:rt + 1], scale=1.0)

        def emit_kvproj(kt3):
            if kt3 == 0:
                xks = xks0
            else:
                xks = xp.tile([P, 8, 384], bf16, name="xks", tag="xks")
                nc.sync.dma_start(
                    xks,
                    xkT_d[:, ts(kt3, 384)].rearrange("(o p) l -> p o l", p=P))
            ps = psS.tile([P, 2, 512], f32, name="ps_s", tag="ps_s")
            for rt in range(2):
                for dc in range(8):
                    nc.tensor.matmul(
                        ps[:, rt, :384],
                        lhsT=wqkT_sb[:, dc, ts(2 + rt, P)],
                        rhs=xks[:, dc, :],
                        start=(dc == 0), stop=(dc == 7),
                    )
            for rt in range(2):
                nc.scalar.activation(
                    kT_sb[:, rt, ts(kt3, 384)], ps[:, rt, :384], Act.Identity,
                    bias=bqk_sb[:, 2 + rt:3 + rt], scale=1.0)
            for l4 in range(3):
                ltv = kt3 * 3 + l4
                psv = psS.tile([P, 2, 512], f32, name="ps_s", tag="ps_s")[:, 0, :256]
                for dc in range(8):
                    nc.tensor.matmul(
                        psv,
                        lhsT=xks[:, dc, ts(l4, P)],
                        rhs=wvT_sb[:, dc, :],
                        start=(dc == 0), stop=(dc == 7),
                    )
                nc.vector.tensor_copy(
                    V_sb[:, ltv, :, 0:64],
                    psv.rearrange("p (h c) -> p h c", c=64),
                )

        # ---- attention stream -------------------------------------------
        avs = {}

        def emit_scores(qt, kt, pair):
            swide = psS.tile([P, 2, 512], f32, name="swide", tag="ps_s")
            for hi in range(2):
                cs = slice(64 * hi, 64 * hi + 64)
                nc.tensor.matmul(
                    swide[:, hi, :],
                    lhsT=kT_sb[cs, pair, ts(kt, P)],
                    rhs=qT_sb[cs, pair, ts(qt, 512)],
                    start=True, stop=True,
                )
            ex1 = expp.tile([P, 2, 512], f16, name="ex1", tag="ex1")
            nc.scalar.activation(ex1, swide, Act.Exp, bias=mln8_c[:, 0:1],
                                 scale=1.0)
            ex2 = expp.tile([P, 2, 512], f16, name="ex2", tag="ex2")
            nc.vector.tensor_mul(
                ex2, ex1, btws[qt][:, kt, 2 * pair:2 * pair + 2, :])
            return ex2

        def emit_av(qt, kt, pair, ex2):
            for hi in range(2):
                h = 2 * pair + hi
                nc.tensor.matmul(
                    avs[qt][pair][:, hi, :],
                    lhsT=V_sb[:, kt, h, :],
                    rhs=ex2[:, hi, :],
                    start=(kt == 0), stop=(kt == KT - 1),
                )

        def emit_step(qt, kt, pair):
            emit_av(qt, kt, pair, emit_scores(qt, kt, pair))

        zrows = {}

        def emit_zrow(qt):
            # Z rows (PSUM row 64 of each AV pair) -> one tiny SBUF tile;
            # one copy per engine so the two run concurrently
            zrow = zp.tile([1, HPC, 512], f16, name="zrow", tag="zrow")
            zrows[qt] = zrow
            nc.scalar.copy(zrow[0:1, 0:2, :], avs[qt][0][64:65, :, :])
            nc.vector.tensor_copy(zrow[0:1, 2:4, :], avs[qt][1][64:65, :, :])

        def emit_norm(qt):
            # broadcast raw Z to 64 partitions with a contraction-1
            # ones-matmul; the reciprocal doubles as the PSUM evacuation;
            # otl muls read the AV PSUM directly (one-PSUM-operand rule)
            zrow = zrows[qt]
            otl = otlp.tile([P, 2, 512], bf16, name="otl", tag="otl")
            for pair in range(2):
                zbc = psS.tile([P, 2, 512], f32, name="zbc", tag="ps_s")
                for hi in range(2):
                    nc.tensor.matmul(
                        zbc[0:64, hi, :],
                        lhsT=ones_c[0:1, :],
                        rhs=zrow[0:1, 2 * pair + hi, :],
                        start=True, stop=True,
                    )
                zrt = zp.tile([64, 2, 512], f32, name="zrt", tag="zrt")
                nc.vector.reciprocal_approx_fast(zrt, zbc[0:64, :, :])
                for hi in range(2):
                    nc.vector.tensor_mul(
                        otl[64 * hi:64 * hi + 64, pair, :],
                        avs[qt][pair][0:64, hi, :], zrt[:, hi, :])
            return otl

        otls = {}

        def emit_proj_pair(qt, jp):
            # out-projection for j-blocks 2*jp, 2*jp+1 of q-block qt
            otl = otls[qt]
            ps = psS.tile([P, 2, 512], f32, name="ps_s", tag="ps_s")
            for j2 in range(2):
                jt = 2 * jp + j2
                for cc in range(2):
                    nc.tensor.matmul(
                        ps[:, j2, :],
                        lhsT=woutT_sb[:, cc, ts(jt, P)],
                        rhs=otl[:, cc, :],
                        start=(cc == 0), stop=(cc == 1),
                    )
            osb = osbs[qt]
            nc.vector.tensor_add(
                osb[:, 2 * jp:2 * jp + 2, :], ps,
                bout_sb[:, 2 * jp:2 * jp + 2, None].to_broadcast((P, 2, 512)))
            if jp == 3:
                nc.sync.dma_start(
                    outT_d[:, ts(qt, 512)].rearrange("(o p) l -> p o l", p=P),
                    osb)

        osbs = {}

        # ---- schedule ---------------------------------------------------
        avs[0] = [psAV.tile([65, 2, 512], f32, name=f"av0{pp}", tag="av")
                  for pp in range(2)]
        emit_qproj(0)
        emit_kvproj(0)
        for kt3 in range(KT3):
            if kt3 + 1 < KT3:
                emit_kvproj(kt3 + 1)
            if kt3 + 1 < 4:
                emit_qproj(kt3 + 1)
            for kt in range(3 * kt3, min(3 * kt3 + 3, KT)):
                for pair in range(2):
                    emit_step(0, kt, pair)
        for lt in range(KT3 + 1, 4):
            emit_qproj(lt)
        woutT_sb = consts.tile([P, 2, D], bf16, name="woutT_sb", tag="woutT_sb")
        nc.sync.dma_start(woutT_sb, woutT_d.rearrange("(o p) m -> p o m", p=P))
        bout_sb = consts.tile([P, 8], f32, name="bout_sb", tag="bout_sb")
        nc.sync.dma_start(bout_sb, bout_d.rearrange("(o p) -> p o", p=P))
        for qt in range(1, 4):
            emit_bias_dma(qt)
            emit_zrow(qt - 1)
            avs[qt] = [psAV.tile([65, 2, 512], f32, name=f"av{qt}{pp}", tag="av")
                       for pp in range(2)]
            osbs[qt - 1] = outp.tile([P, 8, 512], f16, name="osb", tag="osb")
            ex0 = [emit_scores(qt, 0, pair) for pair in range(2)]
            ex1_ = [emit_scores(qt, 1, pair) for pair in range(2)]
            otls[qt - 1] = emit_norm(qt - 1)
            for pair in range(2):
                emit_av(qt, 0, pair, ex0[pair])
            for pair in range(2):
                emit_av(qt, 1, pair, ex1_[pair])
            for kt in range(2, KT):
                for pair in range(2):
                    emit_step(qt, kt, pair)
                if 5 <= kt <= 8:
                    emit_proj_pair(qt - 1, kt - 5)
        osbs[3] = outp.tile([P, 8, 512], f16, name="osb", tag="osb")
        emit_zrow(3)
        otls[3] = emit_norm(3)
        for jp in range(4):
            emit_proj_pair(3, jp)

    nc.compile()
    return nc


def _prep_core_inputs(c, Kp, x, key_padding_mask, attn_bias, W_in, b_in,
                      W_out, b_out):
    b, hg = c // HPC, c % HPC
    hs = slice(256 * hg, 256 * hg + 256)
    f32 = np.float32
    bf16 = ml_dtypes.bfloat16
    KT = Kp // P
    idx = np.where(~key_padding_mask[b])[0]
    nk = len(idx)
    wq, wk, wv = W_in[0:D][hs], W_in[D:2 * D][hs], W_in[2 * D:3 * D][hs]

    xk = np.zeros((Kp, D), dtype=f32)
    xk[:nk] = x[b][idx]

    # bias weights: exp(bias - ln4) fp16, zero at slack slots so the
    # multiply masks them; swizzled [qt, kt, p, h, q] so each partition's
    # DMA read is contiguous per kt chunk.
    full = np.zeros((HPC, Kp, L), dtype=f32)
    full[:, :nk, :] = np.exp(
        attn_bias[b, HPC * hg:HPC * hg + HPC][:, :, idx]
        .transpose(0, 2, 1) - LN4)
    bias8 = np.ascontiguousarray(
        full.reshape(HPC, KT, P, 4, 512).transpose(3, 1, 2, 0, 4)
    ).astype(np.float16)

    return {
        "xT": np.ascontiguousarray(x[b].T).astype(bf16),
        "xkT": np.ascontiguousarray(xk.T).astype(bf16),
        "wqkT": np.ascontiguousarray(
            np.concatenate([wq / 8.0, wk], 0).T).astype(bf16),
        "wvT": np.ascontiguousarray(wv.T).astype(bf16),
        "bqk": np.concatenate([b_in[0:D][hs] / 8.0, b_in[D:2 * D][hs]]).astype(f32),
        "bias8": bias8,
        "woutT": np.ascontiguousarray(W_out[:, hs].T).astype(bf16),
        "bout4": (b_out / float(HPC) + W_out[:, hs] @ b_in[2 * D:3 * D][hs]).astype(f32),
    }


def kernel(x, key_padding_mask, attn_bias, W_in, b_in, W_out, b_out):
    global _compiled, LAST_RESULT
    from concourse.bass_utils import run_bass_kernel_spmd

    nk_max = int((~key_padding_mask).sum(axis=1).max())
    Kp = max(384, -(-nk_max // 384) * 384)

    if _compiled is None or _compiled[0] != Kp:
        _compiled = (Kp, _build(Kp))

    in_maps = [
        _prep_core_inputs(c, Kp, x, key_padding_mask, attn_bias, W_in, b_in,
                          W_out, b_out)
        for c in range(NCORES)
    ]
    res = run_bass_kernel_spmd(
        _compiled[1], in_maps, core_ids=list(range(NCORES)),
        trace_cores=(list(range(NCORES))
                     if os.environ.get("BASS_TRACE") == "1" else None),
    )
    LAST_RESULT = res

    out = np.empty((B, L, D), dtype=np.float32)
    for b in range(B):
        acc = res.results[b * HPC]["outT"].astype(np.float64)
        for g in range(1, HPC):
            acc = acc + res.results[b * HPC + g]["outT"]
        out[b] = acc.T.astype(np.float32)
    return out
